# revision 49
# baseline (speedup 1.0000x reference)
# Cross-attention kernel for Trainium2, 8 NeuronCores.
#
# Sharding: data-parallel over (batch, query-half): core = 2*b + half handles
# batch b, queries [half*1024, (half+1)*1024). No collectives.
#
# On-device layout is feature-major: activations live as [feature, token] in
# fp16. Both layernorms fold into the projections via the postscale form
#   LN(x) @ W'.T = (x @ W' + [-S; bq] x [m; std]) * rstd,
# so the PSUM->SBUF evacuation copy becomes the rstd multiply. The finale
# keeps the prescale form so Gelu reads its PSUM accumulator directly.
#
# v18 structure:
# - All matmul operands are fp16 (fp32r runs in fp32_mode=HIGH at ~1.5
#   cyc/col on HW; fp16 is 1). The finale weights/stats and the K/Q aux rows
#   are fp16; only PSUM accumulation and the DVE stat chains stay fp32.
# - Attention units are c-major so the c=0 finale streams inside the exp
#   window. V and the later K/Q projection chunks stream as PE tasks.
# - Finale LN stats accumulate per-jc into SBUF as each (c,P) column block
#   finishes, so the tail only runs the last jc + Wo chain.
# - All LN stats run K-first in the prefix (kt arrives first) so the ACT
#   engine never reloads the Exp table mid-stream; each finale block costs
#   one Sqrt+Gelu table excursion.
# - DMA: sync queue carries the bulk (K, V, finale weights); scalar triggers
#   only eps/qt/wk/wq/wv then frees ACT for the stats chain; gpsimd stays
#   near-empty for latency-critical rows (aux stats, den rows, O evac, out).
# - Output fp16 (absmax ~5, quantization ~3e-4 abs).
import os
import sys
import tempfile

os.environ["NEURON_COMPILE_CACHE_URL"] = tempfile.mkdtemp(prefix="neff_cache_")
os.environ["AXON_CASSETTE_SALT"] = f"ca-{os.getpid()}-{os.urandom(4).hex()}"

for _p in ("/opt/trn_rl_repo",):
    if os.path.isdir(_p) and _p not in sys.path:
        sys.path.insert(0, _p)

import numpy as np
from contextlib import ExitStack

import concourse.bass as bass
import concourse.tile as tile
from concourse import bacc, mybir
from concourse.bass_utils import run_bass_kernel_spmd

F32 = mybir.dt.float32
F16 = mybir.dt.float16
AF = mybir.ActivationFunctionType
ALU = mybir.AluOpType

B, NQ, NK, D = 4, 2048, 2048, 512
H, DH = 8, 64
NQS = NQ // 2  # queries per core
TEMP = float(np.sqrt(512.0))
LN_EPS = 1e-5
N_CORES = 8

_CACHE = {}
SALT = "v31"
SALT_N = 31


def _build_program():
    nc = bacc.Bacc("TRN2", target_bir_lowering=False, debug=False)

    def din(name, shape, dt=F16):
        return nc.dram_tensor(f"{name}_{SALT}", shape, dt,
                              kind="ExternalInput").ap()

    qt_d = din("qt", [128, 4, NQS])
    kt_d = din("kt", [128, 4, NK])
    vt_d = din("vt", [128, 4, NK])
    wq_d = din("wq", [128, 4, D])
    wk_d = din("wk", [128, 4, D])
    wv_d = din("wv", [128, 4, D])
    wo_d = din("wo", [128, 4, D])
    aq_d = din("aq", [2, D])
    ak_d = din("ak", [2, D])
    ao_d = din("ao", [2, D])
    gb_d = din("gb", [128, 2, 4], F32)
    grow_d = din("grow", [1, D])
    salt_d = din("salt", [1, 8 + SALT_N], F32)
    out_d = nc.dram_tensor(f"out_{SALT}", [128, 4, NQS], F16,
                           kind="ExternalOutput").ap()

    with tile.TileContext(nc) as tc, ExitStack() as top:
        persist = top.enter_context(tc.tile_pool(name="persist", bufs=1))
        qTs = persist.tile([128, 4, NQS], F16)        # projected q
        kTs = persist.tile([128, 4, NK], F16)         # projected k
        vaug = persist.tile([128, 16, 8, 65], F16)    # v natural + ones col
        oTs = persist.tile([128, 4, NQS], F16)        # attention out
        qtin = persist.tile([128, 4, NQS], F16)
        ktin = persist.tile([128, 4, NK], F16)
        vtin = persist.tile([128, 4, NK], F16)
        wq_sb = persist.tile([128, 4, D], F16)
        wk_sb = persist.tile([128, 4, D], F16)
        wv_sb = persist.tile([128, 4, D], F16)
        wo_sb = persist.tile([128, 4, D], F16)
        aq_sb = persist.tile([2, D], F16)
        ak_sb = persist.tile([2, D], F16)
        ao_sb = persist.tile([2, D], F16)
        gb_sb = persist.tile([128, 2, 4], F32)
        grow_sb = persist.tile([1, D], F16)
        auxq = persist.tile([2, NQS], F16)            # [m; std] rows for Q
        auxk = persist.tile([2, NK], F16)
        auxo = persist.tile([2, NQS], F16)            # [m*r; 1] rows, finale
        rstdQ = persist.tile([128, 2, 512], F32)      # replicated 1/std per
        rstdK = persist.tile([128, 4, 512], F32)      # 512-token chunk
        fsum = persist.tile([128, 2, 512], F32)       # finale sum accum
        fssq = persist.tile([128, 2, 512], F32)       # finale ssq accum
        onesh = persist.tile([128, 128], F16)
        eps_t = persist.tile([128, 1], F32)
        wmsrc = persist.tile([128, 512], F16)         # PE warmup fodder

        # DMA queues (only SP/ACT/gpsimd trigger): sync = bulk, scalar =
        # critical small loads only (a 5th+ trigger waits on semaphore
        # recycling, pinning its engine), gpsimd = near-empty for
        # latency-critical mid-stream rows.
        nc.scalar.dma_start(out=eps_t,
                            in_=salt_d[0:1, 0:1].to_broadcast([128, 1]))
        for n2 in range(2):
            ns = slice(512 * n2, 512 * n2 + 512)
            nc.scalar.dma_start(out=qtin[:, :, ns], in_=qt_d[:, :, ns])
        for n2 in range(4):
            ns = slice(512 * n2, 512 * n2 + 512)
            nc.sync.dma_start(out=ktin[:, :, ns], in_=kt_d[:, :, ns])
        nc.scalar.dma_start(out=wk_sb, in_=wk_d)
        nc.scalar.dma_start(out=wq_sb, in_=wq_d)
        nc.scalar.dma_start(out=wv_sb, in_=wv_d)
        for n2 in range(4):
            ns = slice(512 * n2, 512 * n2 + 512)
            nc.sync.dma_start(out=vtin[:, :, ns], in_=vt_d[:, :, ns])
        nc.gpsimd.dma_start(out=aq_sb, in_=aq_d)
        nc.gpsimd.dma_start(out=ak_sb, in_=ak_d)
        nc.sync.dma_start(out=gb_sb, in_=gb_d)
        nc.sync.dma_start(out=grow_sb, in_=grow_d)
        nc.sync.dma_start(out=ao_sb, in_=ao_d)
        nc.sync.dma_start(out=wo_sb, in_=wo_d)
        nc.vector.memset(onesh, 1.0)
        nc.vector.memset(wmsrc, 1.0)
        nc.vector.memset(vaug[:, :, :, 64], 1.0)
        # row 0 is overwritten with m*r in the finale; row 1 stays all-ones
        nc.vector.memset(auxo, 1.0)
        nc.vector.memset(fsum, 0.0)
        nc.vector.memset(fssq, 0.0)

        pmm = top.enter_context(tc.tile_pool(name="pmm", bufs=1, space="PSUM"))
        work = top.enter_context(tc.tile_pool(name="work", bufs=1))
        # Prefix-only stats accumulators, released before the attention
        # pools open.
        pre_ctx = tc.tile_pool(name="pre", bufs=1, space="PSUM")
        pre = pre_ctx.__enter__()

        # PE p-state warmup: the tensor engine only reaches full clock after
        # ~3us of continuous execution; without this, the DMA-gated stats
        # chain runs the whole prefix at half clock. These dummies have no
        # input deps and fill the dead time until kt0 lands.
        warm = pre.tile([128, 512], F32, name="warm", bufs=1)
        for _ in range(6):
            nc.tensor.matmul(warm, onesh, wmsrc, start=True, stop=True)

        def ln_stats(xin, n2, aux, rstd_store, pool):
            """Column LN stats of token chunk n2 of xin [128, 4, *]; fills
            aux rows [m; std] (fp16) and rstd_store[:, n2, :]."""
            ns = slice(512 * n2, 512 * n2 + 512)
            ps_sum = pool.tile([128, 512], F32, name="st", bufs=4)
            for kc in range(4):
                nc.tensor.matmul(ps_sum, onesh, xin[:, kc, ns],
                                 start=(kc == 0), stop=(kc == 3))
            ps_ssq = pool.tile([128, 512], F32, name="st", bufs=4)
            for kc in range(4):
                sq = work.tile([128, 512], F16, name="sq", bufs=3)
                with nc.allow_low_precision("squares in fp16"):
                    nc.vector.tensor_mul(sq, xin[:, kc, ns], xin[:, kc, ns])
                nc.tensor.matmul(ps_ssq, onesh, sq,
                                 start=(kc == 0), stop=(kc == 3))
            m_b = work.tile([128, 512], F16, name="m16", bufs=3)
            with nc.allow_low_precision("fp16 mean"):
                nc.scalar.mul(m_b, ps_sum, 1.0 / 512.0)
            t2 = work.tile([128, 512], F32, name="w32", bufs=6)
            nc.vector.tensor_mul(t2, m_b, ps_sum)
            dv = work.tile([128, 512], F32, name="w32", bufs=6)
            nc.vector.tensor_sub(dv, ps_ssq, t2)
            std_b = work.tile([128, 512], F32, name="w32", bufs=6)
            nc.scalar.activation(std_b, dv, AF.Sqrt, bias=eps_t,
                                 scale=1.0 / 512.0)
            nc.vector.reciprocal_approx_fast(rstd_store[:, n2, :], std_b)
            s16 = work.tile([1, 512], F16, name="s16", bufs=3)
            with nc.allow_low_precision("fp16 std row"):
                nc.vector.tensor_copy(s16, std_b[0:1, :])
            # engine ops cannot move data across partitions; DMA the stat
            # rows into the K=2 aux operand (gpsimd queue: empty, fast).
            nc.gpsimd.dma_start(out=aux[0:1, ns], in_=m_b[0:1, :])
            nc.gpsimd.dma_start(out=aux[1:2, ns], in_=s16)

        def proj_chunk(dst, xin, w_sb, a_sb, aux, rstd_store, jc, n2):
            """dst[:, jc, ns] = (sum_kc w'[:,kc,js].T @ x[:,kc,ns] + aux)*r."""
            ns = slice(512 * n2, 512 * n2 + 512)
            js = slice(128 * jc, 128 * jc + 128)
            pg = pmm.tile([128, 512], F32, name="pmm", bufs=2)
            for kc in range(4):
                nc.tensor.matmul(pg, w_sb[:, kc, js], xin[:, kc, ns],
                                 start=(kc == 0), stop=False)
            nc.tensor.matmul(pg, a_sb[:, js], aux[:, ns],
                             start=False, stop=True)
            with nc.allow_low_precision("fp16 activations"):
                nc.vector.tensor_mul(dst[:, jc, ns], pg, rstd_store[:, n2, :])

        def v_chunk(t):
            ts = slice(128 * t, 128 * t + 128)
            pv = pmm.tile([128, 512], F32, name="pmm", bufs=2)
            for kc in range(4):
                nc.tensor.matmul(pv, vtin[:, kc, ts], wv_sb[:, kc, :],
                                 start=(kc == 0), stop=(kc == 3))
            with nc.allow_low_precision("fp16 activations"):
                nc.vector.tensor_copy(
                    vaug[:, t, :, 0:64],
                    pv.rearrange("p (h v) -> p h v", h=8))

        # ---- Prefix: all LN stats (so ACT never swaps tables mid-exp),
        # then the unit-0-gating projections. The V projections are emitted
        # AFTER the first four attention units (see prologue below) so the
        # exp stream starts while the PE chews through V; V stays out of
        # the window proper since under the ACT exp floor there are only
        # ~5us of PE slack, already spent on K/Q proj streaming. ----
        ln_stats(ktin, 0, auxk, rstdK, pre)
        ln_stats(qtin, 0, auxq, rstdQ, pre)
        ln_stats(ktin, 1, auxk, rstdK, pre)
        ln_stats(qtin, 1, auxq, rstdQ, pre)
        ln_stats(ktin, 2, auxk, rstdK, pre)
        ln_stats(ktin, 3, auxk, rstdK, pre)
        proj_chunk(kTs, ktin, wk_sb, ak_sb, auxk, rstdK, 0, 0)
        proj_chunk(qTs, qtin, wq_sb, aq_sb, auxq, rstdQ, 0, 0)
        pre_ctx.__exit__(None, None, None)  # release stats banks for psL

        # Streaming tasks: remaining projections and the finale stats run
        # through the attention window under the ACT exp stream.
        tasks = {}

        def add_task(u, fn):
            tasks.setdefault(u, []).append(fn)

        def mk_proj(dst, xin, w_sb, a_sb, aux, rstd_store, jc, n2):
            return lambda: proj_chunk(dst, xin, w_sb, a_sb, aux, rstd_store,
                                      jc, n2)

        # K proj (jc=P, n2) due by unit 16*P + 4*n2 - 1; Q proj (jc, n2=c)
        # due by unit 64*c + 16*jc - 1. (jc=0 chunks are emitted with the
        # prologue V block below.)
        kslot = {(1, 0): 20, (1, 1): 26, (1, 2): 32, (1, 3): 36,
                 (2, 0): 52, (2, 1): 58, (2, 2): 64, (2, 3): 68,
                 (3, 0): 84, (3, 1): 88, (3, 2): 92, (3, 3): 96}
        for (jc, n2), u in kslot.items():
            add_task(u, mk_proj(kTs, ktin, wk_sb, ak_sb, auxk, rstdK, jc, n2))
        qslot = {(0, 1): 8, (1, 0): 22, (1, 1): 40, (2, 0): 54,
                 (2, 1): 70, (3, 0): 86, (3, 1): 100}
        for (jc, n2), u in qslot.items():
            add_task(u, mk_proj(qTs, qtin, wq_sb, aq_sb, auxq, rstdQ, jc, n2))

        # ---- Attention: units (c, P, p); c-major so the c=0 finale can
        # stream inside the window. ----
        at_psL = top.enter_context(tc.tile_pool(name="at_psL", bufs=1,
                                                space="PSUM"))
        at_po = top.enter_context(tc.tile_pool(name="at_po", bufs=1,
                                               space="PSUM"))
        at_sb = top.enter_context(tc.tile_pool(name="at_sb", bufs=1))

        units = [(c, P, p) for P in range(4) for c in range(2)
                 for p in range(16)]
        LAG = 4
        pend = {}   # unit idx -> (P, c, p, psO pair, ex tile)

        late = []  # finale-sum closures emitted after the unit loop

        def fin_sums(P, c, nst0, nst1):
            """Accumulate LN sums/ssq of the normalized O column block into
            fsum/fssq (SBUF). Runs as a task ~6 units after the normalize
            chain so the PE never waits on it."""
            cs = slice(512 * c, 512 * c + 512)
            o64 = onesh[0:64, :]
            ps = pmm.tile([128, 512], F32, name="pmm", bufs=2)
            nc.tensor.matmul(ps, o64, nst0, start=True, stop=False)
            nc.tensor.matmul(ps, o64, nst1, start=False, stop=True)
            nc.vector.tensor_add(fsum[:, c, :], fsum[:, c, :], ps)
            sq0 = work.tile([64, 512], F16, name="fsq", bufs=4)
            sq1 = work.tile([64, 512], F16, name="fsq", bufs=4)
            with nc.allow_low_precision("fp16 squares"):
                nc.vector.tensor_mul(sq0, nst0, nst0)
                nc.vector.tensor_mul(sq1, nst1, nst1)
            pq = pmm.tile([128, 512], F32, name="pmm", bufs=2)
            nc.tensor.matmul(pq, o64, sq0, start=True, stop=False)
            nc.tensor.matmul(pq, o64, sq1, start=False, stop=True)
            nc.vector.tensor_add(fssq[:, c, :], fssq[:, c, :], pq)

        def emit_O(u):
            P, c, p, ps_o, ex = pend.pop(u)
            for hh in range(2):
                nc.tensor.matmul(ps_o[hh], vaug[:, p, 2 * P + hh, :],
                                 ex[:, 512 * hh: 512 * hh + 512],
                                 start=(p == 0), stop=(p == 15))
            if p == 15:
                cs = slice(512 * c, 512 * c + 512)
                # Quick DVE copies evacuate both accumulator banks so the
                # next group's first O-matmul isn't blocked; then the two
                # normalize chains run interleaved (DVE and gpsimd overlap).
                oc = [at_sb.tile([65, 512], F32, name=f"oc{hh}", bufs=2)
                      for hh in range(2)]
                den0 = [at_sb.tile([1, 512], F32, name="tl32", bufs=4)
                        for _ in range(2)]
                rr0 = [at_sb.tile([1, 512], F32, name="tl32", bufs=4)
                       for _ in range(2)]
                rrb = [at_sb.tile([64, 512], F32, name="rrb", bufs=3)
                       for _ in range(2)]
                for hh in range(2):
                    nc.vector.tensor_copy(oc[hh], ps_o[hh])
                # recip_approx works from SBUF partition 0: DMA the den
                # rows down, then reciprocal + broadcast.
                for hh in range(2):
                    nc.gpsimd.dma_start(out=den0[hh], in_=oc[hh][64:65, :])
                for hh in range(2):
                    nc.vector.reciprocal_approx_fast(rr0[hh], den0[hh])
                for hh in range(2):
                    nc.gpsimd.partition_broadcast(rrb[hh], rr0[hh])
                ost = at_sb.tile([64, 512], F16, name="ost", bufs=2)
                with nc.allow_low_precision("fp16 storage"):
                    # hh0 rows align with oTs: write in place, no DMA hop.
                    nc.vector.tensor_mul(oTs[0:64, P, cs], oc[0][0:64, :],
                                         rrb[0])
                    nc.vector.tensor_mul(ost, oc[1][0:64, :], rrb[1])
                nc.sync.dma_start(out=oTs[64:128, P, cs], in_=ost)
                nst0 = oTs[0:64, P, cs]
                if u + 15 < len(units):
                    add_task(u + 15,
                             lambda P=P, c=c, n1=ost: fin_sums(P, c, nst0, n1))
                else:
                    late.append(
                        lambda P=P, c=c, n1=ost: fin_sums(P, c, nst0, n1))

        # ---- Finale: LN fold (prescale form) + Wo + gelu + residual.
        # Stats accumulate per-jc into SBUF as each (c, P=jc) column block
        # completes; the Wo chain runs as one clustered block per 512-token
        # chunk (its ACT ops sit consecutively: one Sqrt+Gelu excursion). ----
        def fin_prep(n2):
            """Stats chain for chunk n2: mean/rstd, auxo row, oTs prescale.
            One Sqrt on ACT; everything else DVE."""
            ns = slice(512 * n2, 512 * n2 + 512)
            m_b = work.tile([128, 512], F32, name="w32", bufs=6)
            nc.vector.tensor_scalar_mul(m_b, fsum[:, n2, :], 1.0 / 512.0)
            t2 = work.tile([128, 512], F32, name="w32", bufs=6)
            nc.vector.tensor_mul(t2, m_b, fsum[:, n2, :])
            dv = work.tile([128, 512], F32, name="w32", bufs=6)
            nc.vector.tensor_sub(dv, fssq[:, n2, :], t2)
            std_b = work.tile([128, 512], F32, name="w32", bufs=6)
            nc.scalar.activation(std_b, dv, AF.Sqrt, bias=eps_t,
                                 scale=1.0 / 512.0)
            r_b = work.tile([128, 512], F32, name="rb", bufs=2)
            nc.vector.reciprocal_approx_fast(r_b, std_b)
            with nc.allow_low_precision("fp16 storage"):
                nc.vector.tensor_mul(auxo[0:1, ns], m_b[0:1, :], r_b[0:1, :])
                for jc in range(4):
                    nc.vector.tensor_mul(oTs[:, jc, ns], oTs[:, jc, ns], r_b)

        def fin_out(n2):
            """Wo + gelu + residual for chunk n2 (oTs already prescaled)."""
            ns = slice(512 * n2, 512 * n2 + 512)
            for jc in range(4):
                js = slice(128 * jc, 128 * jc + 128)
                pg = pmm.tile([128, 512], F32, name="pmm", bufs=2)
                for kc in range(4):
                    nc.tensor.matmul(pg, wo_sb[:, kc, js], oTs[:, kc, ns],
                                     start=(kc == 0), stop=False)
                nc.tensor.matmul(pg, ao_sb[:, js], auxo[:, ns],
                                 start=False, stop=True)
                pbm = pmm.tile([128, 512], F32, name="pmm", bufs=2)
                # dummy writes keep the PE p-state up through the gelu/stt
                # latency chain; the real pbm matmul overwrites them
                nc.tensor.matmul(pbm, onesh, wmsrc, start=True, stop=True)
                nc.tensor.matmul(pbm, onesh, wmsrc, start=True, stop=True)
                nc.tensor.matmul(pbm, grow_sb[0:1, js], auxo[0:1, ns],
                                 start=True, stop=True)
                gl = work.tile([128, 512], F32, name="w32", bufs=6)
                nc.scalar.activation(gl, pg, AF.Gelu)
                u2 = work.tile([128, 512], F32, name="w32", bufs=6)
                # u2 = oTs_scaled*g - m*r*g   (oTs already prescaled by r)
                nc.vector.scalar_tensor_tensor(
                    u2, oTs[:, jc, ns], gb_sb[:, 0, jc:jc + 1], pbm,
                    op0=ALU.mult, op1=ALU.subtract)
                of = work.tile([128, 512], F16, name="wof", bufs=4)
                with nc.allow_low_precision("fp16 output"):
                    nc.vector.scalar_tensor_tensor(
                        of, u2, gb_sb[:, 1, jc:jc + 1], gl,
                        op0=ALU.add, op1=ALU.add)
                nc.sync.dma_start(out=out_d[:, jc, ns], in_=of)

        ps_o_cur = None

        def unit_front(u):
            nonlocal ps_o_cur
            c, P, p = units[u]
            if p == 0:
                ps_o_cur = [at_po.tile([65, 512], F32, name=f"po{hh}",
                                       bufs=1) for hh in range(2)]
            psL = at_psL.tile([128, 1024], F32, name="psL", bufs=2)
            ks = slice(128 * p, 128 * p + 128)
            cs = slice(512 * c, 512 * c + 512)
            for hh in range(2):
                rb = 64 * hh
                nc.tensor.matmul(psL[:, 512 * hh: 512 * hh + 512],
                                 kTs[rb:rb + 64, P, ks],
                                 qTs[rb:rb + 64, P, cs],
                                 start=True, stop=True)
            ex = at_sb.tile([128, 1024], F16, name="ex", bufs=LAG + 1)
            nc.scalar.activation(ex, psL, AF.Exp, scale=1.0 / TEMP)
            pend[u] = (P, c, p, ps_o_cur, ex)

        # Prologue: first LAG units' logits+exp go out BEFORE the V block
        # so the ACT exp stream starts while the PE chews through the V
        # projections (emitted here, consumed from unit LAG onward).
        for u in range(LAG):
            unit_front(u)
        for t in range(4):
            v_chunk(t)
        for n2 in range(1, 4):
            proj_chunk(kTs, ktin, wk_sb, ak_sb, auxk, rstdK, 0, n2)
        for t in range(4, 16):
            add_task(t, lambda t=t: v_chunk(t))
        for u in range(LAG, len(units)):
            unit_front(u)
            for fn in tasks.pop(u, ()):
                fn()
            emit_O(u - LAG)
        # ---- Tail: both finale blocks clustered so ACT pays exactly one
        # Sqrt-set and one Gelu-set load (sqrt0, sqrt1, then 8 gelus). ----
        for u in range(len(units) - LAG, len(units)):
            emit_O(u)
        for fn in late:
            fn()
        fin_prep(0)
        fin_prep(1)
        fin_out(0)
        fin_out(1)

    nc.compile()
    return nc


def _chunk_fm(x):
    """[512, N] feature-major -> [128, 4, N] (partition, chunk, col)."""
    n = x.shape[1]
    return np.ascontiguousarray(x.reshape(4, 128, n).transpose(1, 0, 2))


def _prep_inputs(Q, K, V, Wq, Wk, Wv, Wo, g, b, go, bo):
    WqT = np.ascontiguousarray((Wq * g[None, :]).T)
    WkT = np.ascontiguousarray((Wk * g[None, :]).T)
    WvT = np.ascontiguousarray(Wv.T)
    WoT = np.ascontiguousarray((Wo * go[None, :]).T)
    f16 = np.float16
    shared = {
        f"wq_{SALT}": _chunk_fm(WqT).astype(f16),
        f"wk_{SALT}": _chunk_fm(WkT).astype(f16),
        f"wv_{SALT}": _chunk_fm(WvT).astype(f16),
        f"wo_{SALT}": _chunk_fm(WoT).astype(f16),
        f"aq_{SALT}": np.stack([-WqT.sum(0), Wq @ b]).astype(f16),
        f"ak_{SALT}": np.stack([-WkT.sum(0), Wk @ b]).astype(f16),
        f"ao_{SALT}": np.stack([-WoT.sum(0), Wo @ bo]).astype(f16),
        f"gb_{SALT}": np.ascontiguousarray(
            np.stack([go.reshape(4, 128).T, bo.reshape(4, 128).T], axis=1)),
        f"grow_{SALT}": go[None, :].astype(f16),
    }
    in_maps = []
    for core in range(N_CORES):
        bi, half = core // 2, core % 2
        qs = slice(half * NQS, (half + 1) * NQS)
        m = dict(shared)
        m[f"salt_{SALT}"] = np.full((1, 8 + SALT_N), LN_EPS, np.float32)
        m[f"qt_{SALT}"] = _chunk_fm(np.ascontiguousarray(Q[bi, qs, :].T)).astype(f16)
        m[f"kt_{SALT}"] = _chunk_fm(np.ascontiguousarray(K[bi].T)).astype(f16)
        m[f"vt_{SALT}"] = _chunk_fm(np.ascontiguousarray(V[bi].T)).astype(f16)
        in_maps.append(m)
    return in_maps


def kernel(Q, K, V, Wq, Wk, Wv, Wo, ln_qk_g, ln_qk_b, ln_o_g, ln_o_b,
           _trace=False):
    args = [np.asarray(a, dtype=np.float32) for a in
            (Q, K, V, Wq, Wk, Wv, Wo, ln_qk_g, ln_qk_b, ln_o_g, ln_o_b)]
    if "nc" not in _CACHE:
        _CACHE["nc"] = _build_program()
    nc = _CACHE["nc"]
    in_maps = _prep_inputs(*args)
    res = run_bass_kernel_spmd(nc, in_maps, core_ids=list(range(N_CORES)),
                               trace=_trace)
    _CACHE["last_results"] = res
    out = np.empty((B, NQ, D), dtype=np.float32)
    for core in range(N_CORES):
        bi, half = core // 2, core % 2
        o = res.results[core][f"out_{SALT}"].astype(np.float32)  # [128,4,NQS]
        out[bi, half * NQS : (half + 1) * NQS, :] = (
            o.transpose(1, 0, 2).reshape(D, NQS).T)
    return out


# revision 50
# speedup vs baseline: 1.0062x; 1.0062x over previous
# Cross-attention kernel for Trainium2, 8 NeuronCores.
#
# Sharding: data-parallel over (batch, query-half): core = 2*b + half handles
# batch b, queries [half*1024, (half+1)*1024). No collectives.
#
# On-device layout is feature-major: activations live as [feature, token] in
# fp16. Both layernorms fold into the projections via the postscale form
#   LN(x) @ W'.T = (x @ W' + [-S; bq] x [m; std]) * rstd,
# so the PSUM->SBUF evacuation copy becomes the rstd multiply. The finale
# keeps the prescale form so Gelu reads its PSUM accumulator directly.
#
# v18 structure:
# - All matmul operands are fp16 (fp32r runs in fp32_mode=HIGH at ~1.5
#   cyc/col on HW; fp16 is 1). The finale weights/stats and the K/Q aux rows
#   are fp16; only PSUM accumulation and the DVE stat chains stay fp32.
# - Attention units are c-major so the c=0 finale streams inside the exp
#   window. V and the later K/Q projection chunks stream as PE tasks.
# - Finale LN stats accumulate per-jc into SBUF as each (c,P) column block
#   finishes, so the tail only runs the last jc + Wo chain.
# - All LN stats run K-first in the prefix (kt arrives first) so the ACT
#   engine never reloads the Exp table mid-stream; each finale block costs
#   one Sqrt+Gelu table excursion.
# - DMA: sync queue carries the bulk (K, V, finale weights); scalar triggers
#   only eps/qt/wk/wq/wv then frees ACT for the stats chain; gpsimd stays
#   near-empty for latency-critical rows (aux stats, den rows, O evac, out).
# - Output fp16 (absmax ~5, quantization ~3e-4 abs).
import os
import sys
import tempfile

os.environ["NEURON_COMPILE_CACHE_URL"] = tempfile.mkdtemp(prefix="neff_cache_")
os.environ["AXON_CASSETTE_SALT"] = f"ca-{os.getpid()}-{os.urandom(4).hex()}"

for _p in ("/opt/trn_rl_repo",):
    if os.path.isdir(_p) and _p not in sys.path:
        sys.path.insert(0, _p)

import numpy as np
from contextlib import ExitStack

import concourse.bass as bass
import concourse.tile as tile
from concourse import bacc, mybir
from concourse.bass_utils import run_bass_kernel_spmd

F32 = mybir.dt.float32
F16 = mybir.dt.float16
AF = mybir.ActivationFunctionType
ALU = mybir.AluOpType

B, NQ, NK, D = 4, 2048, 2048, 512
H, DH = 8, 64
NQS = NQ // 2  # queries per core
TEMP = float(np.sqrt(512.0))
LN_EPS = 1e-5
N_CORES = 8

_CACHE = {}
SALT = "v32"
SALT_N = 32


def _build_program():
    nc = bacc.Bacc("TRN2", target_bir_lowering=False, debug=False)

    def din(name, shape, dt=F16):
        return nc.dram_tensor(f"{name}_{SALT}", shape, dt,
                              kind="ExternalInput").ap()

    qt_d = din("qt", [128, 4, NQS])
    kt_d = din("kt", [128, 4, NK])
    vt_d = din("vt", [128, 4, NK])
    wq_d = din("wq", [128, 4, D])
    wk_d = din("wk", [128, 4, D])
    wv_d = din("wv", [128, 4, D])
    wo_d = din("wo", [128, 4, D])
    aq_d = din("aq", [2, D])
    ak_d = din("ak", [2, D])
    ao_d = din("ao", [2, D])
    gb_d = din("gb", [128, 2, 4], F32)
    grow_d = din("grow", [1, D])
    salt_d = din("salt", [1, 8 + SALT_N], F32)
    out_d = nc.dram_tensor(f"out_{SALT}", [128, 4, NQS], F16,
                           kind="ExternalOutput").ap()

    with tile.TileContext(nc) as tc, ExitStack() as top:
        persist = top.enter_context(tc.tile_pool(name="persist", bufs=1))
        qTs = persist.tile([128, 4, NQS], F16)        # projected q
        kTs = persist.tile([128, 4, NK], F16)         # projected k
        vaug = persist.tile([128, 16, 8, 65], F16)    # v natural + ones col
        oTs = persist.tile([128, 4, NQS], F16)        # attention out
        qtin = persist.tile([128, 4, NQS], F16)
        ktin = persist.tile([128, 4, NK], F16)
        vtin = persist.tile([128, 4, NK], F16)
        wq_sb = persist.tile([128, 4, D], F16)
        wk_sb = persist.tile([128, 4, D], F16)
        wv_sb = persist.tile([128, 4, D], F16)
        wo_sb = persist.tile([128, 4, D], F16)
        aq_sb = persist.tile([2, D], F16)
        ak_sb = persist.tile([2, D], F16)
        ao_sb = persist.tile([2, D], F16)
        gb_sb = persist.tile([128, 2, 4], F32)
        grow_sb = persist.tile([1, D], F16)
        auxq = persist.tile([2, NQS], F16)            # [m; std] rows for Q
        auxk = persist.tile([2, NK], F16)
        auxo = persist.tile([2, NQS], F16)            # [m*r; 1] rows, finale
        rstdQ = persist.tile([128, 2, 512], F32)      # replicated 1/std per
        rstdK = persist.tile([128, 4, 512], F32)      # 512-token chunk
        fsum = persist.tile([128, 2, 512], F32)       # finale sum accum
        fssq = persist.tile([128, 2, 512], F32)       # finale ssq accum
        onesh = persist.tile([128, 128], F16)
        eps_t = persist.tile([128, 1], F32)
        wmsrc = persist.tile([128, 512], F16)         # PE warmup fodder

        # DMA queues (only SP/ACT/gpsimd trigger): sync = bulk, scalar =
        # critical small loads only (a 5th+ trigger waits on semaphore
        # recycling, pinning its engine), gpsimd = near-empty for
        # latency-critical mid-stream rows.
        nc.scalar.dma_start(out=eps_t,
                            in_=salt_d[0:1, 0:1].to_broadcast([128, 1]))
        for n2 in range(2):
            ns = slice(512 * n2, 512 * n2 + 512)
            nc.scalar.dma_start(out=qtin[:, :, ns], in_=qt_d[:, :, ns])
        for n2 in range(4):
            ns = slice(512 * n2, 512 * n2 + 512)
            nc.sync.dma_start(out=ktin[:, :, ns], in_=kt_d[:, :, ns])
        nc.scalar.dma_start(out=wk_sb, in_=wk_d)
        nc.scalar.dma_start(out=wq_sb, in_=wq_d)
        nc.scalar.dma_start(out=wv_sb, in_=wv_d)
        for n2 in range(4):
            ns = slice(512 * n2, 512 * n2 + 512)
            nc.sync.dma_start(out=vtin[:, :, ns], in_=vt_d[:, :, ns])
        nc.gpsimd.dma_start(out=aq_sb, in_=aq_d)
        nc.gpsimd.dma_start(out=ak_sb, in_=ak_d)
        nc.sync.dma_start(out=gb_sb, in_=gb_d)
        nc.sync.dma_start(out=grow_sb, in_=grow_d)
        nc.sync.dma_start(out=ao_sb, in_=ao_d)
        nc.sync.dma_start(out=wo_sb, in_=wo_d)
        nc.vector.memset(onesh, 1.0)
        nc.vector.memset(wmsrc, 1.0)
        nc.vector.memset(vaug[:, :, :, 64], 1.0)
        # row 0 is overwritten with m*r in the finale; row 1 stays all-ones
        nc.vector.memset(auxo, 1.0)
        nc.vector.memset(fsum, 0.0)
        nc.vector.memset(fssq, 0.0)

        pmm = top.enter_context(tc.tile_pool(name="pmm", bufs=1, space="PSUM"))
        work = top.enter_context(tc.tile_pool(name="work", bufs=1))
        # Prefix-only stats accumulators, released before the attention
        # pools open.
        pre_ctx = tc.tile_pool(name="pre", bufs=1, space="PSUM")
        pre = pre_ctx.__enter__()

        # PE p-state warmup: the tensor engine only reaches full clock after
        # ~3us of continuous execution; without this, the DMA-gated stats
        # chain runs the whole prefix at half clock. These dummies have no
        # input deps and fill the dead time until kt0 lands.
        warm = pre.tile([128, 512], F32, name="warm", bufs=1)
        for _ in range(6):
            nc.tensor.matmul(warm, onesh, wmsrc, start=True, stop=True)

        def ln_stats(xin, n2, aux, rstd_store, pool):
            """Column LN stats of token chunk n2 of xin [128, 4, *]; fills
            aux rows [m; std] (fp16) and rstd_store[:, n2, :]."""
            ns = slice(512 * n2, 512 * n2 + 512)
            ps_sum = pool.tile([128, 512], F32, name="st", bufs=4)
            for kc in range(4):
                nc.tensor.matmul(ps_sum, onesh, xin[:, kc, ns],
                                 start=(kc == 0), stop=(kc == 3))
            ps_ssq = pool.tile([128, 512], F32, name="st", bufs=4)
            for kc in range(4):
                sq = work.tile([128, 512], F16, name="sq", bufs=3)
                with nc.allow_low_precision("squares in fp16"):
                    nc.vector.tensor_mul(sq, xin[:, kc, ns], xin[:, kc, ns])
                nc.tensor.matmul(ps_ssq, onesh, sq,
                                 start=(kc == 0), stop=(kc == 3))
            m_b = work.tile([128, 512], F16, name="m16", bufs=3)
            with nc.allow_low_precision("fp16 mean"):
                nc.scalar.mul(m_b, ps_sum, 1.0 / 512.0)
            t2 = work.tile([128, 512], F32, name="w32", bufs=6)
            nc.vector.tensor_mul(t2, m_b, ps_sum)
            dv = work.tile([128, 512], F32, name="w32", bufs=6)
            nc.vector.tensor_sub(dv, ps_ssq, t2)
            std_b = work.tile([128, 512], F32, name="w32", bufs=6)
            nc.scalar.activation(std_b, dv, AF.Sqrt, bias=eps_t,
                                 scale=1.0 / 512.0)
            nc.vector.reciprocal_approx_fast(rstd_store[:, n2, :], std_b)
            s16 = work.tile([1, 512], F16, name="s16", bufs=3)
            with nc.allow_low_precision("fp16 std row"):
                nc.vector.tensor_copy(s16, std_b[0:1, :])
            # engine ops cannot move data across partitions; DMA the stat
            # rows into the K=2 aux operand (gpsimd queue: empty, fast).
            nc.gpsimd.dma_start(out=aux[0:1, ns], in_=m_b[0:1, :])
            nc.gpsimd.dma_start(out=aux[1:2, ns], in_=s16)

        def proj_chunk(dst, xin, w_sb, a_sb, aux, rstd_store, jc, n2):
            """dst[:, jc, ns] = (sum_kc w'[:,kc,js].T @ x[:,kc,ns] + aux)*r."""
            ns = slice(512 * n2, 512 * n2 + 512)
            js = slice(128 * jc, 128 * jc + 128)
            pg = pmm.tile([128, 512], F32, name="pmm", bufs=2)
            for kc in range(4):
                nc.tensor.matmul(pg, w_sb[:, kc, js], xin[:, kc, ns],
                                 start=(kc == 0), stop=False)
            nc.tensor.matmul(pg, a_sb[:, js], aux[:, ns],
                             start=False, stop=True)
            with nc.allow_low_precision("fp16 activations"):
                nc.vector.tensor_mul(dst[:, jc, ns], pg, rstd_store[:, n2, :])

        def v_chunk(t):
            ts = slice(128 * t, 128 * t + 128)
            pv = pmm.tile([128, 512], F32, name="pmm", bufs=2)
            for kc in range(4):
                nc.tensor.matmul(pv, vtin[:, kc, ts], wv_sb[:, kc, :],
                                 start=(kc == 0), stop=(kc == 3))
            with nc.allow_low_precision("fp16 activations"):
                nc.vector.tensor_copy(
                    vaug[:, t, :, 0:64],
                    pv.rearrange("p (h v) -> p h v", h=8))

        # ---- Prefix: all LN stats (so ACT never swaps tables mid-exp),
        # then the unit-0-gating projections. The V projections are emitted
        # AFTER the first four attention units (see prologue below) so the
        # exp stream starts while the PE chews through V; V stays out of
        # the window proper since under the ACT exp floor there are only
        # ~5us of PE slack, already spent on K/Q proj streaming. ----
        ln_stats(ktin, 0, auxk, rstdK, pre)
        ln_stats(qtin, 0, auxq, rstdQ, pre)
        ln_stats(ktin, 1, auxk, rstdK, pre)
        ln_stats(qtin, 1, auxq, rstdQ, pre)
        ln_stats(ktin, 2, auxk, rstdK, pre)
        ln_stats(ktin, 3, auxk, rstdK, pre)
        proj_chunk(kTs, ktin, wk_sb, ak_sb, auxk, rstdK, 0, 0)
        proj_chunk(qTs, qtin, wq_sb, aq_sb, auxq, rstdQ, 0, 0)
        pre_ctx.__exit__(None, None, None)  # release stats banks for psL

        # Streaming tasks: remaining projections and the finale stats run
        # through the attention window under the ACT exp stream.
        tasks = {}

        def add_task(u, fn):
            tasks.setdefault(u, []).append(fn)

        def mk_proj(dst, xin, w_sb, a_sb, aux, rstd_store, jc, n2):
            return lambda: proj_chunk(dst, xin, w_sb, a_sb, aux, rstd_store,
                                      jc, n2)

        # K proj (jc=P, n2) due by unit 16*P + 4*n2 - 1; Q proj (jc, n2=c)
        # due by unit 64*c + 16*jc - 1. (jc=0 chunks are emitted with the
        # prologue V block below.)
        kslot = {(1, 0): 20, (1, 1): 26, (1, 2): 32, (1, 3): 36,
                 (2, 0): 52, (2, 1): 58, (2, 2): 64, (2, 3): 68,
                 (3, 0): 84, (3, 1): 88, (3, 2): 92, (3, 3): 96}
        for (jc, n2), u in kslot.items():
            add_task(u, mk_proj(kTs, ktin, wk_sb, ak_sb, auxk, rstdK, jc, n2))
        qslot = {(0, 1): 8, (1, 0): 22, (1, 1): 40, (2, 0): 54,
                 (2, 1): 70, (3, 0): 86, (3, 1): 100}
        for (jc, n2), u in qslot.items():
            add_task(u, mk_proj(qTs, qtin, wq_sb, aq_sb, auxq, rstdQ, jc, n2))

        # ---- Attention: units (c, P, p); c-major so the c=0 finale can
        # stream inside the window. ----
        at_psL = top.enter_context(tc.tile_pool(name="at_psL", bufs=1,
                                                space="PSUM"))
        at_po = top.enter_context(tc.tile_pool(name="at_po", bufs=1,
                                               space="PSUM"))
        at_sb = top.enter_context(tc.tile_pool(name="at_sb", bufs=1))

        units = [(c, P, p) for P in range(4) for c in range(2)
                 for p in range(16)]
        LAG = 4
        pend = {}   # unit idx -> (P, c, p, psO pair, ex tile)

        late = []  # finale-sum closures emitted after the unit loop

        def fin_sums(P, c, nst0, nst1):
            """Accumulate LN sums/ssq of the normalized O column block into
            fsum/fssq (SBUF). Runs as a task ~6 units after the normalize
            chain so the PE never waits on it."""
            cs = slice(512 * c, 512 * c + 512)
            o64 = onesh[0:64, :]
            ps = pmm.tile([128, 512], F32, name="pmm", bufs=2)
            nc.tensor.matmul(ps, o64, nst0, start=True, stop=False)
            nc.tensor.matmul(ps, o64, nst1, start=False, stop=True)
            nc.vector.tensor_add(fsum[:, c, :], fsum[:, c, :], ps)
            sq0 = work.tile([64, 512], F16, name="fsq", bufs=4)
            sq1 = work.tile([64, 512], F16, name="fsq", bufs=4)
            with nc.allow_low_precision("fp16 squares"):
                nc.vector.tensor_mul(sq0, nst0, nst0)
                nc.vector.tensor_mul(sq1, nst1, nst1)
            pq = pmm.tile([128, 512], F32, name="pmm", bufs=2)
            nc.tensor.matmul(pq, o64, sq0, start=True, stop=False)
            nc.tensor.matmul(pq, o64, sq1, start=False, stop=True)
            nc.vector.tensor_add(fssq[:, c, :], fssq[:, c, :], pq)

        def emit_O(u):
            P, c, p, ps_o, ex = pend.pop(u)
            for hh in range(2):
                nc.tensor.matmul(ps_o[hh], vaug[:, p, 2 * P + hh, :],
                                 ex[:, 512 * hh: 512 * hh + 512],
                                 start=(p == 0), stop=(p == 15))
            if p == 15:
                cs = slice(512 * c, 512 * c + 512)
                # Quick DVE copies evacuate both accumulator banks so the
                # next group's first O-matmul isn't blocked; then the two
                # normalize chains run interleaved (DVE and gpsimd overlap).
                oc = [at_sb.tile([65, 512], F32, name=f"oc{hh}", bufs=2)
                      for hh in range(2)]
                den0 = [at_sb.tile([1, 512], F32, name="tl32", bufs=4)
                        for _ in range(2)]
                rr0 = [at_sb.tile([1, 512], F32, name="tl32", bufs=4)
                       for _ in range(2)]
                rrb = [at_sb.tile([64, 512], F32, name="rrb", bufs=3)
                       for _ in range(2)]
                for hh in range(2):
                    nc.vector.tensor_copy(oc[hh], ps_o[hh])
                # recip_approx works from SBUF partition 0: DMA the den
                # rows down, then reciprocal + broadcast.
                for hh in range(2):
                    nc.gpsimd.dma_start(out=den0[hh], in_=oc[hh][64:65, :])
                for hh in range(2):
                    nc.vector.reciprocal_approx_fast(rr0[hh], den0[hh])
                for hh in range(2):
                    nc.gpsimd.partition_broadcast(rrb[hh], rr0[hh])
                ost = at_sb.tile([64, 512], F16, name="ost", bufs=2)
                with nc.allow_low_precision("fp16 storage"):
                    # hh0 rows align with oTs: write in place, no DMA hop.
                    nc.vector.tensor_mul(oTs[0:64, P, cs], oc[0][0:64, :],
                                         rrb[0])
                    nc.vector.tensor_mul(ost, oc[1][0:64, :], rrb[1])
                nc.sync.dma_start(out=oTs[64:128, P, cs], in_=ost)
                nst0 = oTs[0:64, P, cs]
                if u + 15 < len(units):
                    add_task(u + 15,
                             lambda P=P, c=c, n1=ost: fin_sums(P, c, nst0, n1))
                else:
                    late.append(
                        lambda P=P, c=c, n1=ost: fin_sums(P, c, nst0, n1))

        # ---- Finale: LN fold (prescale form) + Wo + gelu + residual.
        # Stats accumulate per-jc into SBUF as each (c, P=jc) column block
        # completes; the Wo chain runs as one clustered block per 512-token
        # chunk (its ACT ops sit consecutively: one Sqrt+Gelu excursion). ----
        def fin_prep(n2):
            """Stats chain for chunk n2: mean/rstd, auxo row, oTs prescale.
            One Sqrt on ACT; everything else DVE."""
            ns = slice(512 * n2, 512 * n2 + 512)
            m_b = work.tile([128, 512], F32, name="w32", bufs=6)
            nc.vector.tensor_scalar_mul(m_b, fsum[:, n2, :], 1.0 / 512.0)
            t2 = work.tile([128, 512], F32, name="w32", bufs=6)
            nc.vector.tensor_mul(t2, m_b, fsum[:, n2, :])
            dv = work.tile([128, 512], F32, name="w32", bufs=6)
            nc.vector.tensor_sub(dv, fssq[:, n2, :], t2)
            std_b = work.tile([128, 512], F32, name="w32", bufs=6)
            nc.scalar.activation(std_b, dv, AF.Sqrt, bias=eps_t,
                                 scale=1.0 / 512.0)
            r_b = work.tile([128, 512], F32, name="rb", bufs=2)
            nc.vector.reciprocal_approx_fast(r_b, std_b)
            with nc.allow_low_precision("fp16 storage"):
                nc.vector.tensor_mul(auxo[0:1, ns], m_b[0:1, :], r_b[0:1, :])
                for jc in range(4):
                    nc.vector.tensor_mul(oTs[:, jc, ns], oTs[:, jc, ns], r_b)

        def fin_out(n2):
            """Wo + gelu + residual for chunk n2 (oTs already prescaled)."""
            ns = slice(512 * n2, 512 * n2 + 512)
            for jc in range(4):
                js = slice(128 * jc, 128 * jc + 128)
                pg = pmm.tile([128, 512], F32, name="pmm", bufs=2)
                for kc in range(4):
                    nc.tensor.matmul(pg, wo_sb[:, kc, js], oTs[:, kc, ns],
                                     start=(kc == 0), stop=False)
                nc.tensor.matmul(pg, ao_sb[:, js], auxo[:, ns],
                                 start=False, stop=True)
                pbm = pmm.tile([128, 512], F32, name="pmm", bufs=2)
                # dummy writes keep the PE p-state up through the gelu/stt
                # latency chain; the real pbm matmul overwrites them
                nc.tensor.matmul(pbm, onesh, wmsrc, start=True, stop=True)
                nc.tensor.matmul(pbm, onesh, wmsrc, start=True, stop=True)
                nc.tensor.matmul(pbm, grow_sb[0:1, js], auxo[0:1, ns],
                                 start=True, stop=True)
                gl = work.tile([128, 512], F32, name="w32", bufs=6)
                nc.scalar.activation(gl, pg, AF.Gelu)
                u2 = work.tile([128, 512], F32, name="w32", bufs=6)
                # u2 = oTs_scaled*g - m*r*g   (oTs already prescaled by r)
                nc.vector.scalar_tensor_tensor(
                    u2, oTs[:, jc, ns], gb_sb[:, 0, jc:jc + 1], pbm,
                    op0=ALU.mult, op1=ALU.subtract)
                of = work.tile([128, 512], F16, name="wof", bufs=4)
                with nc.allow_low_precision("fp16 output"):
                    nc.vector.scalar_tensor_tensor(
                        of, u2, gb_sb[:, 1, jc:jc + 1], gl,
                        op0=ALU.add, op1=ALU.add)
                nc.sync.dma_start(out=out_d[:, jc, ns], in_=of)

        ps_o_cur = None

        def unit_front(u):
            nonlocal ps_o_cur
            c, P, p = units[u]
            if p == 0:
                ps_o_cur = [at_po.tile([65, 512], F32, name=f"po{hh}",
                                       bufs=1) for hh in range(2)]
            psL = at_psL.tile([128, 1024], F32, name="psL", bufs=2)
            ks = slice(128 * p, 128 * p + 128)
            cs = slice(512 * c, 512 * c + 512)
            for hh in range(2):
                rb = 64 * hh
                nc.tensor.matmul(psL[:, 512 * hh: 512 * hh + 512],
                                 kTs[rb:rb + 64, P, ks],
                                 qTs[rb:rb + 64, P, cs],
                                 start=True, stop=True)
            ex = at_sb.tile([128, 1024], F16, name="ex", bufs=LAG + 1)
            nc.scalar.activation(ex, psL, AF.Exp, scale=1.0 / TEMP)
            pend[u] = (P, c, p, ps_o_cur, ex)

        # Prologue: first LAG units' logits+exp go out BEFORE the V block
        # so the ACT exp stream starts while the PE chews through the V
        # projections (emitted here, consumed from unit LAG onward).
        for u in range(LAG):
            unit_front(u)
        for t in range(16):
            v_chunk(t)
        for n2 in range(1, 4):
            proj_chunk(kTs, ktin, wk_sb, ak_sb, auxk, rstdK, 0, n2)
        for u in range(LAG, len(units)):
            unit_front(u)
            for fn in tasks.pop(u, ()):
                fn()
            emit_O(u - LAG)
        # ---- Tail: both finale blocks clustered so ACT pays exactly one
        # Sqrt-set and one Gelu-set load (sqrt0, sqrt1, then 8 gelus). ----
        for u in range(len(units) - LAG, len(units)):
            emit_O(u)
        for fn in late:
            fn()
        fin_prep(0)
        fin_prep(1)
        fin_out(0)
        fin_out(1)

    nc.compile()
    return nc


def _chunk_fm(x):
    """[512, N] feature-major -> [128, 4, N] (partition, chunk, col)."""
    n = x.shape[1]
    return np.ascontiguousarray(x.reshape(4, 128, n).transpose(1, 0, 2))


def _prep_inputs(Q, K, V, Wq, Wk, Wv, Wo, g, b, go, bo):
    WqT = np.ascontiguousarray((Wq * g[None, :]).T)
    WkT = np.ascontiguousarray((Wk * g[None, :]).T)
    WvT = np.ascontiguousarray(Wv.T)
    WoT = np.ascontiguousarray((Wo * go[None, :]).T)
    f16 = np.float16
    shared = {
        f"wq_{SALT}": _chunk_fm(WqT).astype(f16),
        f"wk_{SALT}": _chunk_fm(WkT).astype(f16),
        f"wv_{SALT}": _chunk_fm(WvT).astype(f16),
        f"wo_{SALT}": _chunk_fm(WoT).astype(f16),
        f"aq_{SALT}": np.stack([-WqT.sum(0), Wq @ b]).astype(f16),
        f"ak_{SALT}": np.stack([-WkT.sum(0), Wk @ b]).astype(f16),
        f"ao_{SALT}": np.stack([-WoT.sum(0), Wo @ bo]).astype(f16),
        f"gb_{SALT}": np.ascontiguousarray(
            np.stack([go.reshape(4, 128).T, bo.reshape(4, 128).T], axis=1)),
        f"grow_{SALT}": go[None, :].astype(f16),
    }
    in_maps = []
    for core in range(N_CORES):
        bi, half = core // 2, core % 2
        qs = slice(half * NQS, (half + 1) * NQS)
        m = dict(shared)
        m[f"salt_{SALT}"] = np.full((1, 8 + SALT_N), LN_EPS, np.float32)
        m[f"qt_{SALT}"] = _chunk_fm(np.ascontiguousarray(Q[bi, qs, :].T)).astype(f16)
        m[f"kt_{SALT}"] = _chunk_fm(np.ascontiguousarray(K[bi].T)).astype(f16)
        m[f"vt_{SALT}"] = _chunk_fm(np.ascontiguousarray(V[bi].T)).astype(f16)
        in_maps.append(m)
    return in_maps


def kernel(Q, K, V, Wq, Wk, Wv, Wo, ln_qk_g, ln_qk_b, ln_o_g, ln_o_b,
           _trace=False):
    args = [np.asarray(a, dtype=np.float32) for a in
            (Q, K, V, Wq, Wk, Wv, Wo, ln_qk_g, ln_qk_b, ln_o_g, ln_o_b)]
    if "nc" not in _CACHE:
        _CACHE["nc"] = _build_program()
    nc = _CACHE["nc"]
    in_maps = _prep_inputs(*args)
    res = run_bass_kernel_spmd(nc, in_maps, core_ids=list(range(N_CORES)),
                               trace=_trace)
    _CACHE["last_results"] = res
    out = np.empty((B, NQ, D), dtype=np.float32)
    for core in range(N_CORES):
        bi, half = core // 2, core % 2
        o = res.results[core][f"out_{SALT}"].astype(np.float32)  # [128,4,NQS]
        out[bi, half * NQS : (half + 1) * NQS, :] = (
            o.transpose(1, 0, 2).reshape(D, NQS).T)
    return out
